# revision 6
# baseline (speedup 1.0000x reference)
"""BachNet beam-search inference kernel for 8 TRN2 NeuronCores.

Strategy (single NEFF launch, tensor-parallel over the hidden dim):
  - N == P == 62, so stage-1's sort only reorders rows; stages are computed in
    natural pitch order and the one-hot concatenations become row-slices /
    row-gathers of the first-layer weight matrices.
  - Each core owns a 256-wide column shard of every w1/w2 and a 256-row shard
    of every w3.  x @ w1 mat-vecs run on VectorE (fused mul+acc), the batched
    layer-2 GEMMs on TensorE.  One AllGather per stage shares hidden
    activations; layer-3 logits are produced as partial sums + AllReduce.
  - The stage-2 top-62 selection runs fully on-device and replicated:
    gpsimd kth_largest gives the exact 63rd-largest threshold, triangular
    matmuls turn the mask into row-major compaction ranks, and a gpsimd
    local_scatter builds the alto-pitch one-hot.
  - The final (stage-3) top-62 + sort runs on host from the tiny [62,62]
    result matrices (exact, matches jnp.argsort tie-breaking).
  - selu is computed as lam*relu(v) + lam*alpha*(exp(min(v,0))-1) with the
    lam factor pre-folded into the layer-1/2 weights on host.
"""
import sys

sys.path.insert(0, "/opt/trn_rl_repo")

import numpy as np
import ml_dtypes

import concourse.bacc as bacc
import concourse.tile as tile
import concourse.mybir as mybir
from concourse import bass_utils

P = 62           # pitch classes == num candidates
D = 10112        # bass input dim (= 79 * 128)
H = 2048         # hidden
NCORES = 8
HS = H // NCORES          # 256 hidden columns per core
KT1 = D // 128            # 79 k-tiles for layer 1
KT2 = H // 128            # 16 k-tiles for layer 2
MT = HS // 128            # 2 m-tiles per core shard
LAM = 1.0507009873554805
ALPHA = 1.6732632423543772
LA = LAM * ALPHA

f32 = mybir.dt.float32
bf16 = mybir.dt.bfloat16
i16 = mybir.dt.int16
OP = mybir.AluOpType
AX = mybir.AxisListType
AF = mybir.ActivationFunctionType

# layer-1 k-tile chunks for DMA streaming (~1.25 MiB each)
_CHUNKS = []
_t = 0
while _t < KT1:
    _n = min(10, KT1 - _t)
    _CHUNKS.append((_t, _n))
    _t += _n


def _build():
    nc = bacc.Bacc("TRN2", target_bir_lowering=False, debug=False,
                   num_devices=NCORES)

    def din(name, shape, dtype=f32):
        return nc.dram_tensor(name, shape, dtype, kind="ExternalInput")

    xT_d = din("xT", [128, KT1])
    w1_d = {s: din(f"{s}w1i", [128, KT1 * HS]) for s in "bat"}
    w2_d = {s: din(f"{s}w2i", [128, KT2 * HS]) for s in "bat"}
    w3_d = {s: din(f"{s}w3r", [128, MT * P]) for s in "bat"}
    aohT_d = din("aohT", [128, MT * P])
    tohb_d = din("tohb", [P, HS])
    toha_d = din("toha", [P, HS])
    b1_d = {s: din(f"{s}b1c", [128, MT]) for s in "bat"}
    b2_d = {s: din(f"{s}b2r", [1, HS]) for s in "bat"}
    b3_d = {s: din(f"{s}b3r", [1, P]) for s in "bat"}
    ident_d = din("ident", [128, 128])
    LT_d = din("LTc", [P, P])
    SLT_d = din("SLTc", [P, P])
    iotaF_d = din("iotaF", [P, P])
    iotaC_d = din("iotaC", [P, 1])
    onesR_d = din("onesR", [1, HS])
    onesC_d = din("onesC", [128, 1])
    onesCbf_d = din("onesCbf", [P, 1], bf16)
    iotaFbf_d = din("iotaFbf", [64, P], bf16)
    warm_d = din("warm", [16, 32])

    pa_out = nc.dram_tensor("pa_out", [P, P], f32, kind="ExternalOutput")
    pt_out = nc.dram_tensor("pt_out", [P, P], f32, kind="ExternalOutput")

    with tile.TileContext(nc) as tc:
        with (
            tc.tile_pool(name="consts", bufs=1) as cp,
            tc.tile_pool(name="stream", bufs=4) as sp,
            tc.tile_pool(name="work", bufs=1) as wp,
            tc.tile_pool(name="trans", bufs=3) as tp,
            tc.tile_pool(name="pmv", bufs=1, space="PSUM") as pp_mv,
            tc.tile_pool(name="ptp", bufs=2, space="PSUM") as pp_tp,
            tc.tile_pool(name="pl1", bufs=2, space="PSUM") as pp_l1,
            tc.tile_pool(name="psel", bufs=2, space="PSUM") as pp_sel,
            tc.tile_pool(name="dram", bufs=1, space="DRAM") as dp,
        ):
            def cload(src, shape, dtype=f32):
                t = cp.tile(shape, dtype, tag=src.name, name="c_" + src.name)
                nc.sync.dma_start(t[:], src[:])
                return t

            # --- constants / small inputs ---
            xTs = cload(xT_d, [128, KT1])
            idn = cload(ident_d, [128, 128])
            lt = cload(LT_d, [P, P])
            slt = cload(SLT_d, [P, P])
            iof = cload(iotaF_d, [P, P])
            ioc = cload(iotaC_d, [P, 1])
            onr = cload(onesR_d, [1, HS])
            onc = cload(onesC_d, [128, 1])
            ocb = cload(onesCbf_d, [P, 1], bf16)
            iofb = cload(iotaFbf_d, [64, P], bf16)
            aohT = cload(aohT_d, [128, MT * P])
            tohb = cload(tohb_d, [P, HS])
            toha = cload(toha_d, [P, HS])
            b1s = {s: cload(b1_d[s], [128, MT]) for s in "bat"}
            b2s = {s: cload(b2_d[s], [1, HS]) for s in "bat"}
            b3s = {s: cload(b3_d[s], [1, P]) for s in "bat"}

            # --- warmup collective (absorbs first-collective latency) ---
            warm_sb = wp.tile([16, 32], f32, tag="warm")
            nc.gpsimd.dma_start(warm_sb[:], warm_d[:])
            wbi = dp.tile([16, 32], f32, tag="wbi")
            wbo = dp.tile([128, 32], f32, tag="wbo")
            nc.gpsimd.dma_start(wbi[:], warm_sb[:])
            nc.gpsimd.collective_compute(
                "AllGather", OP.bypass, replica_groups=[list(range(NCORES))],
                ins=[wbi[:].opt()], outs=[wbo[:].opt()])
            wg = wp.tile([128, 32], f32, tag="warm2")
            nc.gpsimd.dma_start(wg[:], wbo[:])

            # --- layer-1 mat-vec on VectorE: sh = lam*(x @ w1[:, cols] + b1)
            def matvec(s):
                acc = wp.tile([128, HS], f32, tag=f"acc_{s}", name=f"acc_{s}")
                first = True
                for (t0, tn) in _CHUNKS:
                    ck = sp.tile([128, 10 * HS], f32, tag="w1ck", name="w1ck")
                    nc.sync.dma_start(ck[:, :tn * HS],
                                      w1_d[s][:, t0 * HS:(t0 + tn) * HS])
                    for t in range(tn):
                        sl = ck[:, t * HS:(t + 1) * HS]
                        xsc = xTs[:, t0 + t:t0 + t + 1]
                        if first:
                            nc.vector.tensor_scalar(acc[:], sl, xsc, None,
                                                    OP.mult)
                            first = False
                        else:
                            nc.vector.scalar_tensor_tensor(
                                acc[:], sl, xsc, acc[:], OP.mult, OP.add)
                psh = pp_mv.tile([1, HS], f32, tag="mv", name="psh")
                nc.tensor.matmul(psh[:], onc[:, 0:1], acc[:], start=True,
                                 stop=True)
                shrow = tp.tile([1, HS], f32, tag="shrow", name="shrow")
                nc.vector.tensor_copy(shrow[:], psh[:])
                cols = []
                for mt in range(MT):
                    ptp = pp_tp.tile([128, 1], f32, tag="tp", name="ptp")
                    nc.tensor.transpose(ptp[:],
                                        shrow[:1, mt * 128:(mt + 1) * 128],
                                        idn[:1, :1])
                    scol = wp.tile([128, 1], f32, tag=f"shc_{s}{mt}", name=f"shc_{s}{mt}")
                    nc.vector.tensor_add(scol[:], ptp[:], b1s[s][:, mt:mt + 1])
                    cols.append(scol)
                return cols

            # selu: dst = lam*relu(pre) + lam*alpha*(exp(min(pre,0))-1)
            # (lam is pre-folded into pre; shcol optionally added first)
            def selu_chain(pre_ap, shcol, parts, width, tag):
                shp = [parts, width]
                m = tp.tile(shp, f32, tag="selu_m", name="selu_m")
                r = tp.tile(shp, f32, tag="selu_r", name="selu_r")
                e = tp.tile(shp, f32, tag="selu_e", name="selu_e")
                e2 = tp.tile(shp, f32, tag="selu_e2", name="selu_e2")
                dst = wp.tile(shp, f32, tag=tag, name=tag)
                if shcol is None:
                    nc.vector.tensor_scalar(m[:], pre_ap, 0.0, None, OP.min)
                    nc.vector.tensor_scalar(r[:], pre_ap, 0.0, None, OP.max)
                else:
                    nc.vector.tensor_scalar(m[:], pre_ap, shcol, 0.0, OP.add,
                                            OP.min)
                    nc.vector.tensor_scalar(r[:], pre_ap, shcol, 0.0, OP.add,
                                            OP.max)
                nc.scalar.activation(e[:], m[:], AF.Exp, scale=1.0 / LAM)
                nc.vector.tensor_scalar(e2[:], e[:], LA, -LA, OP.mult, OP.add)
                nc.vector.tensor_add(dst[:], r[:], e2[:])
                return dst

            # ---------------- stage 1 + 2 (bass || alto) ----------------
            shb = matvec("b")
            sha = matvec("a")

            # w2 / w3 shards needed right after the first AllGather
            w2s = {}
            w3s = {}
            for s in "ba":
                w2s[s] = cp.tile([128, KT2 * HS], f32, tag=f"w2_{s}", name=f"w2_{s}")
                nc.sync.dma_start(w2s[s][:], w2_d[s][:])
                w3s[s] = cp.tile([128, MT * P], f32, tag=f"w3_{s}", name=f"w3_{s}")
                nc.sync.dma_start(w3s[s][:], w3_d[s][:])

            # bass h1 tiles [128,1]; alto h1 tiles [128,62]
            h1b = [selu_chain(shb[mt][:], None, 128, 1, f"h1b{mt}")
                   for mt in range(MT)]
            h1a = [selu_chain(aohT[:, mt * P:(mt + 1) * P], shb_a, 128, P,
                              f"h1a{mt}")
                   for mt, shb_a in enumerate([sha[0][:], sha[1][:]])]

            # fused AllGather of [256, 63] (alto cols 0..61, bass col 62)
            W1 = P + 1
            hb1 = dp.tile([HS, W1], f32, tag="hb1")
            for mt in range(MT):
                nc.gpsimd.dma_start(hb1[mt * 128:(mt + 1) * 128, 0:P],
                                    h1a[mt][:])
                nc.gpsimd.dma_start(hb1[mt * 128:(mt + 1) * 128, P:W1],
                                    h1b[mt][:])
            ghb1 = dp.tile([H, W1], f32, tag="ghb1")
            nc.gpsimd.collective_compute(
                "AllGather", OP.bypass, replica_groups=[list(range(NCORES))],
                ins=[hb1[:].opt()], outs=[ghb1[:].opt()])
            H1T = wp.tile([128, KT2 * W1], f32, tag="H1T")
            nc.gpsimd.dma_start(
                H1T[:].rearrange("p (kt w) -> p kt w", w=W1),
                ghb1[:].rearrange("(kt p) w -> p kt w", p=128))

            # layer 2 (alto [128,62] x2, bass [128,1] x2) + selu
            h2a = []
            h2b = []
            for mt in range(MT):
                pya = pp_l1.tile([128, P], f32, tag="l2", name="pya")
                for kt in range(KT2):
                    nc.tensor.matmul(
                        pya[:],
                        w2s["a"][:, kt * HS + mt * 128:kt * HS + (mt + 1) * 128],
                        H1T[:, kt * W1:kt * W1 + P],
                        start=(kt == 0), stop=False)
                nc.tensor.matmul(pya[:], b2s["a"][:1, mt * 128:(mt + 1) * 128],
                                 onr[:1, :P], start=False, stop=True)
                h2a.append(selu_chain(pya[:], None, 128, P, f"h2a{mt}"))
                pyb = pp_tp.tile([128, 1], f32, tag="tp", name="pyb")
                for kt in range(KT2):
                    nc.tensor.matmul(
                        pyb[:],
                        w2s["b"][:, kt * HS + mt * 128:kt * HS + (mt + 1) * 128],
                        H1T[:, kt * W1 + P:kt * W1 + W1],
                        start=(kt == 0), stop=False)
                nc.tensor.matmul(pyb[:], b2s["b"][:1, mt * 128:(mt + 1) * 128],
                                 onr[:1, :1], start=False, stop=True)
                h2b.append(selu_chain(pyb[:], None, 128, 1, f"h2b{mt}"))

            # partial layer-3 logits (row-sharded w3) -> AllReduce
            plg_a = pp_sel.tile([P, P], f32, tag="ps")
            for mt in range(MT):
                nc.tensor.matmul(plg_a[:], h2a[mt][:],
                                 w3s["a"][:, mt * P:(mt + 1) * P],
                                 start=(mt == 0), stop=False)
            nc.tensor.matmul(plg_a[:], onr[:1, :P], b3s["a"][:1, :],
                             start=False, stop=True)
            plg_b = pp_tp.tile([1, P], f32, tag="tp")
            for mt in range(MT):
                nc.tensor.matmul(plg_b[:], h2b[mt][:],
                                 w3s["b"][:, mt * P:(mt + 1) * P],
                                 start=(mt == 0), stop=False)
            nc.tensor.matmul(plg_b[:], onr[:1, :1], b3s["b"][:1, :],
                             start=False, stop=True)
            NR = 65   # rows 0..61 alto, row 64 bass (32-aligned base)
            lgcat = wp.tile([NR, P], f32, tag="lgcat")
            nc.vector.memset(lgcat[:], 0.0)
            nc.vector.tensor_copy(lgcat[:P, :], plg_a[:])
            nc.vector.tensor_copy(lgcat[64:NR, :], plg_b[:])
            lgb = dp.tile([NR, P], f32, tag="lgb")
            nc.gpsimd.dma_start(lgb[:], lgcat[:])
            lgr = dp.tile([NR, P], f32, tag="lgr")
            nc.gpsimd.collective_compute(
                "AllReduce", OP.add, replica_groups=[list(range(NCORES))],
                ins=[lgb[:].opt()], outs=[lgr[:].opt()])
            S = wp.tile([NR, P], f32, tag="S")
            nc.gpsimd.dma_start(S[:], lgr[:])

            # fused softmax over the 62 alto candidate rows + the bass row
            nm = wp.tile([NR, 1], f32, tag="nm")
            nc.vector.tensor_reduce(nm[:], S[:], axis=AX.X, op=OP.max,
                                    negate=True)
            E = wp.tile([NR, P], f32, tag="E")
            ssum = wp.tile([NR, 1], f32, tag="ssum")
            nc.scalar.activation(E[:], S[:], AF.Exp, bias=nm[:],
                                 accum_out=ssum[:])
            rec = wp.tile([NR, 1], f32, tag="rec")
            nc.vector.reciprocal(rec[:], ssum[:])
            # p column: transpose bass exp row, scale by its softmax denom
            erow = wp.tile([1, P], f32, tag="erow")
            nc.vector.tensor_copy(erow[:], E[64:NR, :])
            rc62 = wp.tile([1, 1], f32, tag="rc62")
            nc.vector.tensor_copy(rc62[:], rec[64:NR, 0:1])
            ptp2 = pp_tp.tile([P, 1], f32, tag="tp")
            nc.tensor.transpose(ptp2[:], erow[:1, :], idn[:1, :1])
            pbc = pp_tp.tile([P, 1], f32, tag="tp")
            nc.tensor.matmul(pbc[:], onr[:1, :P], rc62[:1, :1],
                             start=True, stop=True)
            v1 = wp.tile([P, 1], f32, tag="v1")
            nc.vector.tensor_mul(v1[:], ptp2[:], rec[:P, :])
            v = wp.tile([P, 1], f32, tag="v")
            nc.vector.tensor_mul(v[:], v1[:], pbc[:])
            # anchor the warmup collective so it isn't dead code
            nc.vector.scalar_tensor_tensor(v[:], wg[:P, 0:1], 0.0, v[:],
                                           OP.mult, OP.add)
            PA = wp.tile([P, P], f32, tag="PA")
            nc.vector.tensor_scalar(PA[:], E[:P, :], v[:], None, OP.mult)
            nc.sync.dma_start(pa_out[:], PA[:])

            # ---------------- on-device top-62 selection ----------------
            padded = wp.tile([128, P], f32, tag="padded")
            nc.vector.memset(padded[:], -1e30)
            nc.vector.tensor_copy(padded[:P, :], PA[:])
            kth = wp.tile([1, 2], f32, tag="kth")
            nc.gpsimd.kth_largest(kth[:], padded[:], n_per_lane=P, k=128,
                                  quantile=1.0 - 61.5 / 3843.0)
            pt63 = pp_tp.tile([P, 1], f32, tag="tp")
            nc.tensor.matmul(pt63[:], onr[:1, :P], kth[:1, 1:2], start=True,
                             stop=True)
            t63 = wp.tile([P, 1], f32, tag="t63")
            nc.vector.tensor_copy(t63[:], pt63[:])
            mask = wp.tile([P, P], f32, tag="mask")
            nc.vector.tensor_scalar(mask[:], PA[:], t63[:], None, OP.is_gt)
            pmT = pp_sel.tile([P, P], f32, tag="ps")
            nc.tensor.transpose(pmT[:], mask[:], idn[:P, :P])
            mT = wp.tile([P, P], f32, tag="mT")
            nc.vector.tensor_copy(mT[:], pmT[:])
            prc = pp_sel.tile([P, P], f32, tag="ps")
            nc.tensor.matmul(prc[:], mT[:], lt[:], start=True, stop=True)
            rc = wp.tile([P, P], f32, tag="rc")
            nc.vector.tensor_copy(rc[:], prc[:])
            pro = pp_tp.tile([1, P], f32, tag="tp")
            nc.tensor.matmul(pro[:], rc[:, P - 1:P], slt[:], start=True,
                             stop=True)
            ror = wp.tile([1, P], f32, tag="ror")
            nc.vector.tensor_copy(ror[:], pro[:])
            proc = pp_tp.tile([P, 1], f32, tag="tp")
            nc.tensor.transpose(proc[:], ror[:1, :], idn[:1, :1])
            roc = wp.tile([P, 1], f32, tag="roc")
            nc.vector.tensor_copy(roc[:], proc[:])
            re_ = wp.tile([P, 1], f32, tag="re")
            nc.vector.tensor_add(re_[:], roc[:], rc[:, P - 1:P])
            g1 = tp.tile([P, P], f32, tag="selu_m")
            nc.vector.tensor_scalar(g1[:], iof[:], roc[:], None, OP.is_ge)
            g2 = tp.tile([P, P], f32, tag="selu_r")
            nc.vector.tensor_scalar(g2[:], iof[:], re_[:], None, OP.is_lt)
            bb = wp.tile([P, P], f32, tag="bb")
            nc.vector.tensor_mul(bb[:], g1[:], g2[:])
            # alto one-hot via local_scatter of column indices by rank
            t1 = tp.tile([P, P], f32, tag="selu_e")
            nc.vector.tensor_scalar(t1[:], rc[:], roc[:], None, OP.add)
            t2 = tp.tile([P, P], f32, tag="selu_e2")
            nc.vector.tensor_mul(t2[:], t1[:], mask[:])
            t3 = tp.tile([P, P], f32, tag="selu_m")
            nc.vector.tensor_scalar(t3[:], t2[:], -1.0, None, OP.add)
            idx = wp.tile([64, P], i16, tag="idx")
            nc.vector.memset(idx[:], -1)
            nc.vector.tensor_copy(idx[:P, :], t3[:])
            sc = wp.tile([64, 64], bf16, tag="sc")
            nc.gpsimd.local_scatter(sc[:], iofb[:], idx[:], channels=64,
                                    num_elems=64, num_idxs=P)
            pas = pp_tp.tile([1, P], f32, tag="tp")
            nc.tensor.matmul(pas[:], ocb[:], sc[:P, :P], start=True, stop=True)
            asr = wp.tile([1, P], f32, tag="asr")
            nc.vector.tensor_copy(asr[:], pas[:])
            pab = pp_sel.tile([P, P], f32, tag="ps")
            nc.tensor.matmul(pab[:], onr[:1, :P], asr[:1, :], start=True,
                             stop=True)
            ba = wp.tile([P, P], f32, tag="ba")
            nc.vector.tensor_scalar(ba[:], pab[:], ioc[:], None, OP.is_equal)
            # probs of the selected candidates, in rank order
            pz = pp_sel.tile([P, P], f32, tag="ps")
            nc.tensor.matmul(pz[:], bb[:], PA[:], start=True, stop=True)
            pbat = pp_sel.tile([P, P], f32, tag="ps")
            nc.tensor.transpose(pbat[:], ba[:], idn[:P, :P])
            bat = wp.tile([P, P], f32, tag="bat")
            nc.vector.tensor_copy(bat[:], pbat[:])
            pm = tp.tile([P, P], f32, tag="selu_r")
            nc.vector.tensor_mul(pm[:], pz[:], bat[:])
            pcol = wp.tile([P, 1], f32, tag="pcol")
            nc.vector.tensor_reduce(pcol[:], pm[:], axis=AX.X, op=OP.add)

            # ---------------- stage 3 (tenor) ----------------
            sht = matvec("t")
            w2s["t"] = cp.tile([128, KT2 * HS], f32, tag="w2_t", name="w2_t")
            nc.sync.dma_start(w2s["t"][:], w2_d["t"][:])
            w3s["t"] = cp.tile([128, MT * P], f32, tag="w3_t", name="w3_t")
            nc.sync.dma_start(w3s["t"][:], w3_d["t"][:])

            h1t = []
            for mt in range(MT):
                pg = pp_l1.tile([128, P], f32, tag="l2", name="pg")
                nc.tensor.matmul(pg[:], tohb[:, mt * 128:(mt + 1) * 128],
                                 bb[:], start=True, stop=False)
                nc.tensor.matmul(pg[:], toha[:, mt * 128:(mt + 1) * 128],
                                 ba[:], start=False, stop=True)
                h1t.append(selu_chain(pg[:], sht[mt][:], 128, P, f"h1t{mt}"))

            ht1 = dp.tile([HS, P], f32, tag="ht1")
            for mt in range(MT):
                nc.gpsimd.dma_start(ht1[mt * 128:(mt + 1) * 128, :],
                                    h1t[mt][:])
            ght = dp.tile([H, P], f32, tag="ght")
            nc.gpsimd.collective_compute(
                "AllGather", OP.bypass, replica_groups=[list(range(NCORES))],
                ins=[ht1[:].opt()], outs=[ght[:].opt()])
            H1tT = wp.tile([128, KT2 * P], f32, tag="H1tT")
            nc.gpsimd.dma_start(
                H1tT[:].rearrange("p (kt w) -> p kt w", w=P),
                ght[:].rearrange("(kt p) w -> p kt w", p=128))

            h2t = []
            for mt in range(MT):
                pyt = pp_l1.tile([128, P], f32, tag="l2", name="pyt")
                for kt in range(KT2):
                    nc.tensor.matmul(
                        pyt[:],
                        w2s["t"][:, kt * HS + mt * 128:kt * HS + (mt + 1) * 128],
                        H1tT[:, kt * P:(kt + 1) * P],
                        start=(kt == 0), stop=False)
                nc.tensor.matmul(pyt[:], b2s["t"][:1, mt * 128:(mt + 1) * 128],
                                 onr[:1, :P], start=False, stop=True)
                h2t.append(selu_chain(pyt[:], None, 128, P, f"h2t{mt}"))

            plg_t = pp_sel.tile([P, P], f32, tag="ps")
            for mt in range(MT):
                nc.tensor.matmul(plg_t[:], h2t[mt][:],
                                 w3s["t"][:, mt * P:(mt + 1) * P],
                                 start=(mt == 0), stop=False)
            nc.tensor.matmul(plg_t[:], onr[:1, :P], b3s["t"][:1, :],
                             start=False, stop=True)
            lgt_sb = wp.tile([P, P], f32, tag="lgt_sb")
            nc.vector.tensor_copy(lgt_sb[:], plg_t[:])
            lgt = dp.tile([P, P], f32, tag="lgt")
            nc.gpsimd.dma_start(lgt[:], lgt_sb[:])
            lgtr = dp.tile([P, P], f32, tag="lgtr")
            nc.gpsimd.collective_compute(
                "AllReduce", OP.add, replica_groups=[list(range(NCORES))],
                ins=[lgt[:].opt()], outs=[lgtr[:].opt()])
            S3 = wp.tile([P, P], f32, tag="S3")
            nc.gpsimd.dma_start(S3[:], lgtr[:])

            nm3 = wp.tile([P, 1], f32, tag="nm3")
            nc.vector.tensor_reduce(nm3[:], S3[:], axis=AX.X, op=OP.max,
                                    negate=True)
            E3 = wp.tile([P, P], f32, tag="E3")
            ssum3 = wp.tile([P, 1], f32, tag="ssum3")
            nc.scalar.activation(E3[:], S3[:], AF.Exp, bias=nm3[:],
                                 accum_out=ssum3[:])
            rec3 = wp.tile([P, 1], f32, tag="rec3")
            nc.vector.reciprocal(rec3[:], ssum3[:])
            vv = wp.tile([P, 1], f32, tag="vv")
            nc.vector.tensor_mul(vv[:], rec3[:], pcol[:])
            PT = wp.tile([P, P], f32, tag="PT")
            nc.vector.tensor_scalar(PT[:], E3[:], vv[:], None, OP.mult)
            nc.sync.dma_start(pt_out[:], PT[:])

    nc.compile()
    return nc


_NC_CACHE = None


def _get_nc():
    global _NC_CACHE
    if _NC_CACHE is None:
        _NC_CACHE = _build()
    return _NC_CACHE


def _prep_inputs(inputs):
    lam = np.float32(LAM)
    x = np.asarray(inputs["inputs_bass"], np.float32)
    xT = np.ascontiguousarray(x.reshape(KT1, 128).T)

    def w1img(w):
        # [D, 256] -> [128, 79*256]
        return np.ascontiguousarray(
            w.reshape(KT1, 128, HS).transpose(1, 0, 2).reshape(128, KT1 * HS))

    def w2img(w):
        return np.ascontiguousarray(
            w.reshape(KT2, 128, HS).transpose(1, 0, 2).reshape(128, KT2 * HS))

    def mtimg(w):
        # [256, P] -> [128, MT*P]
        return np.ascontiguousarray(
            w.reshape(MT, 128, P).transpose(1, 0, 2).reshape(128, MT * P))

    base = {
        "ident": np.eye(128, dtype=np.float32),
        "LTc": (np.arange(P)[:, None] <= np.arange(P)[None, :]).astype(np.float32),
        "SLTc": (np.arange(P)[:, None] < np.arange(P)[None, :]).astype(np.float32),
        "iotaF": np.broadcast_to(np.arange(P, dtype=np.float32), (P, P)).copy(),
        "iotaC": np.arange(P, dtype=np.float32)[:, None].copy(),
        "onesR": np.ones((1, HS), np.float32),
        "onesC": np.ones((128, 1), np.float32),
        "onesCbf": np.ones((P, 1), ml_dtypes.bfloat16),
        "iotaFbf": np.broadcast_to(
            np.arange(P, dtype=ml_dtypes.bfloat16), (64, P)).copy(),
        "warm": np.zeros((16, 32), np.float32),
        "xT": xT,
    }
    names = {"b": "b", "a": "a", "t": "t"}
    W = {k: np.asarray(v, np.float32) for k, v in inputs.items()}
    in_maps = []
    for c in range(NCORES):
        cols = slice(HS * c, HS * (c + 1))
        m = dict(base)
        for s in names:
            w1 = W[f"{s}w1"]
            m[f"{s}w1i"] = w1img(lam * w1[:D, cols])
            m[f"{s}w2i"] = w2img(lam * W[f"{s}w2"][:, cols])
            m[f"{s}w3r"] = mtimg(W[f"{s}w3"][cols, :])
            m[f"{s}b1c"] = np.ascontiguousarray(
                (lam * W[f"{s}b1"][cols]).reshape(MT, 128).T)
            m[f"{s}b2r"] = (lam * W[f"{s}b2"][cols])[None, :].copy()
            m[f"{s}b3r"] = (W[f"{s}b3"] / NCORES)[None, :].copy()
        aw1 = W["aw1"]
        m["aohT"] = mtimg(np.ascontiguousarray((lam * aw1[D:D + P, cols]).T))
        tw1 = W["tw1"]
        m["tohb"] = np.ascontiguousarray(lam * tw1[D:D + P, cols])
        m["toha"] = np.ascontiguousarray(lam * tw1[D + P:D + 2 * P, cols])
        in_maps.append(m)
    return in_maps


def _postprocess(pa, pt):
    flat = pa.reshape(-1)
    order = np.argsort(-flat, kind="stable")[:P]
    sel = np.sort(order)                  # device rank order = flat position
    j_sel = sel // P
    a_sel = sel % P
    flat3 = pt.reshape(-1)
    idx3 = np.argsort(-flat3, kind="stable")[:P]
    row = idx3 // P
    out = np.stack([
        flat3[idx3],
        j_sel[row].astype(np.float32),
        a_sel[row].astype(np.float32),
        (idx3 % P).astype(np.float32),
    ], axis=1)
    return out


def run(inputs, trace=False):
    nc = _get_nc()
    in_maps = _prep_inputs(inputs)
    res = bass_utils.run_bass_kernel_spmd(
        nc, in_maps, core_ids=list(range(NCORES)), trace=trace)
    r0 = res.results[0]
    out = _postprocess(r0["pa_out"], r0["pt_out"])
    return out, res.exec_time_ns


def kernel(**inputs) -> np.ndarray:
    out, _ = run(inputs, trace=False)
    return out


# revision 12
# speedup vs baseline: 1.0580x; 1.0580x over previous
"""BachNet beam-search inference kernel for 8 TRN2 NeuronCores.

Strategy (single NEFF launch, tensor-parallel over the hidden dim):
  - N == P == 62, so stage-1's sort only reorders rows; stages are computed in
    natural pitch order and the one-hot concatenations become row-slices /
    row-gathers of the first-layer weight matrices.
  - Each core owns a 256-wide column shard of every w1/w2; w3 is replicated.
    The x @ w1 mat-vecs run on VectorE as fused multiply-reduce over
    transposed weight images; the batched layer-2 GEMMs run on TensorE.
    One AllGather shares layer-1 activations, a second shares layer-2
    activations (logits are then computed locally from replicated w3).
  - The stage-2 top-62 selection runs fully on-device and replicated: a
    3-round 62-ary probe search (ScalarE sign-count against a broadcast
    copy of the flattened scores) finds a threshold with exactly 62
    elements above it; triangular matmuls turn the mask into row-major
    compaction ranks, and a gpsimd local_scatter builds the alto one-hot.
  - The final (stage-3) top-62 + sort runs on host from the tiny [62,62]
    result matrices (exact, matches jnp.argsort tie-breaking).
  - selu is computed as lam*relu(v) + lam*alpha*(exp(min(v,0))-1) with the
    lam factor pre-folded into the layer-1/2 weights on host.
"""
import sys

sys.path.insert(0, "/opt/trn_rl_repo")

import numpy as np
import ml_dtypes

import concourse.bacc as bacc
import concourse.tile as tile
import concourse.mybir as mybir
from concourse import bass_utils

P = 62           # pitch classes == num candidates
D = 10112        # bass input dim (= 79 * 128)
H = 2048         # hidden
NCORES = 8
HS = H // NCORES          # 256 hidden columns per core
KT2 = H // 128            # 16 k-tiles for layer 2
MT = HS // 128            # 2 m-tiles per core shard
CHK = 2528                # layer-1 k-chunk (D = 4*2528)
NCH = D // CHK            # 4 chunks per h-tile
LAM = 1.0507009873554805
ALPHA = 1.6732632423543772
LA = LAM * ALPHA
FLAT = P * P              # 3844

f32 = mybir.dt.float32
bf16 = mybir.dt.bfloat16
i16 = mybir.dt.int16
OP = mybir.AluOpType
AX = mybir.AxisListType
AF = mybir.ActivationFunctionType
RG = [list(range(NCORES))]


def _build():
    nc = bacc.Bacc("TRN2", target_bir_lowering=False, debug=False,
                   num_devices=NCORES)

    def din(name, shape, dtype=f32):
        return nc.dram_tensor(name, shape, dtype, kind="ExternalInput")

    xr_d = din("xr", [1, D])
    w1_d = {s: din(f"{s}w1t", [128, MT * D]) for s in "bat"}
    w2_d = {s: din(f"{s}w2i", [128, KT2 * HS]) for s in "bat"}
    w3_d = {s: din(f"{s}w3i", [128, KT2 * P]) for s in "bat"}
    aohT_d = din("aohT", [128, MT * P])
    tohb_d = din("tohb", [P, HS])
    toha_d = din("toha", [P, HS])
    b1_d = {s: din(f"{s}b1c", [128, MT]) for s in "bat"}
    b2_d = {s: din(f"{s}b2r", [1, HS]) for s in "bat"}
    b3_d = {s: din(f"{s}b3r", [1, P]) for s in "bat"}
    ident_d = din("ident", [128, 128])
    LT_d = din("LTc", [P, P])
    SLT_d = din("SLTc", [P, P])
    iotaF_d = din("iotaF", [P, P])
    iotaC_d = din("iotaC", [P, 1])
    iotaC1_d = din("iotaC1", [P, 1])
    onesR_d = din("onesR", [1, HS])
    onesCbf_d = din("onesCbf", [P, 1], bf16)
    iotaFbf_d = din("iotaFbf", [64, P], bf16)
    warm_d = din("warm", [16, 32])

    pa_out = nc.dram_tensor("pa_out", [P, P], f32, kind="ExternalOutput")
    pt_out = nc.dram_tensor("pt_out", [P, P], f32, kind="ExternalOutput")

    with tile.TileContext(nc) as tc:
        with (
            tc.tile_pool(name="consts", bufs=1) as cp,
            tc.tile_pool(name="stream", bufs=2) as sp,
            tc.tile_pool(name="mvscr", bufs=2) as scrp,
            tc.tile_pool(name="work", bufs=1) as wp,
            tc.tile_pool(name="trans", bufs=3) as tp,
            tc.tile_pool(name="ptp", bufs=2, space="PSUM") as pp_tp,
            tc.tile_pool(name="pl1", bufs=2, space="PSUM") as pp_l1,
            tc.tile_pool(name="psel", bufs=2, space="PSUM") as pp_sel,
            tc.tile_pool(name="dram", bufs=1, space="DRAM") as dp,
        ):
            def cload(src, shape, dtype=f32, eng=None):
                t = cp.tile(shape, dtype, tag=src.name, name="c_" + src.name)
                (eng or nc.sync).dma_start(t[:], src[:])
                return t

            # --- small constants (sync queue, ahead of the weight stream) ---
            idn = cload(ident_d, [128, 128])
            lt = cload(LT_d, [P, P])
            slt = cload(SLT_d, [P, P])
            iof = cload(iotaF_d, [P, P])
            ioc = cload(iotaC_d, [P, 1])
            ioc1 = cload(iotaC1_d, [P, 1])
            onr = cload(onesR_d, [1, HS])
            ocb = cload(onesCbf_d, [P, 1], bf16)
            iofb = cload(iotaFbf_d, [64, P], bf16)
            b1s = {s: cload(b1_d[s], [128, MT]) for s in "bat"}
            b2s = {s: cload(b2_d[s], [1, HS]) for s in "bat"}
            b3s = {s: cload(b3_d[s], [1, P]) for s in "bat"}

            # --- warmup collective + bulk small weights on gpsimd ---
            warm_sb = wp.tile([16, 32], f32, tag="warm")
            nc.gpsimd.dma_start(warm_sb[:], warm_d[:])
            wbi = dp.tile([16, 32], f32, tag="wbi")
            wbo = dp.tile([128, 32], f32, tag="wbo")
            nc.gpsimd.dma_start(wbi[:], warm_sb[:])
            nc.gpsimd.collective_compute(
                "AllGather", OP.bypass, replica_groups=RG,
                ins=[wbi[:].opt()], outs=[wbo[:].opt()])
            wg = wp.tile([128, 32], f32, tag="warm2")
            nc.gpsimd.dma_start(wg[:], wbo[:])

            # --- x broadcast [128, D] via step-0 DMA from DRAM ---
            xb = wp.tile([128, D], f32, tag="xb")
            nc.gpsimd.dma_start(
                xb[:],
                xr_d[:].rearrange("a b -> (a b)")[None, :].broadcast_to(
                    [128, D]))

            aohT = cload(aohT_d, [128, MT * P], eng=nc.gpsimd)
            tohb = cload(tohb_d, [P, HS], eng=nc.gpsimd)
            toha = cload(toha_d, [P, HS], eng=nc.gpsimd)
            w2s = {s: cload(w2_d[s], [128, KT2 * HS], eng=nc.gpsimd)
                   for s in "bat"}
            w3s = {s: cload(w3_d[s], [128, KT2 * P], eng=nc.gpsimd)
                   for s in "bat"}


            # --- layer-1 mat-vec: fused mul+reduce over [128, D] rows ---
            # sh[h] = lam * (x @ w1[:, col_h] + b1[col_h]); w1t image rows = h
            def matvec(s):
                # per chunk: VectorE elementwise product, ScalarE free-axis
                # accumulate (Identity activation with accum_out); the two
                # engines pipeline chunk-to-chunk under the DMA stream.
                cols = []
                for mt in range(MT):
                    accs = [wp.tile([128, 1], f32, tag=f"ac_{s}{mt}{i}",
                                    name=f"ac_{s}{mt}{i}")
                            for i in range(NCH)]
                    for ci in range(NCH):
                        ck = sp.tile([128, CHK], f32, tag="w1ck", name="w1ck")
                        nc.sync.dma_start(
                            ck[:],
                            w1_d[s][:, mt * D + ci * CHK:mt * D + (ci + 1) * CHK])
                        prod = scrp.tile([128, CHK], f32, tag="mvscr",
                                         name="mvscr")
                        nc.vector.tensor_mul(prod[:], ck[:],
                                             xb[:, ci * CHK:(ci + 1) * CHK])
                        nc.scalar.activation(prod[:], prod[:], AF.Identity,
                                             accum_out=accs[ci][:])
                    p01 = tp.tile([128, 1], f32, tag="mvp0", name="p01")
                    nc.vector.tensor_add(p01[:], accs[0][:], accs[1][:])
                    p23 = tp.tile([128, 1], f32, tag="mvp1", name="p23")
                    nc.vector.tensor_add(p23[:], accs[2][:], accs[3][:])
                    p03 = tp.tile([128, 1], f32, tag="mvp2", name="p03")
                    nc.vector.tensor_add(p03[:], p01[:], p23[:])
                    scol = wp.tile([128, 1], f32, tag=f"shc_{s}{mt}",
                                   name=f"shc_{s}{mt}")
                    nc.vector.tensor_add(scol[:], p03[:],
                                         b1s[s][:, mt:mt + 1])
                    cols.append(scol)
                return cols

            # selu: dst = lam*relu(pre) + lam*alpha*(exp(min(pre,0))-1)
            def selu_chain(pre_ap, shcol, parts, width, tag):
                shp = [parts, width]
                m = tp.tile(shp, f32, tag="selu_m", name="selu_m")
                r = tp.tile(shp, f32, tag="selu_r", name="selu_r")
                e = tp.tile(shp, f32, tag="selu_e", name="selu_e")
                e2 = tp.tile(shp, f32, tag="selu_e2", name="selu_e2")
                dst = wp.tile(shp, f32, tag=tag, name=tag)
                if shcol is None:
                    nc.vector.tensor_scalar(m[:], pre_ap, 0.0, None, OP.min)
                    nc.vector.tensor_scalar(r[:], pre_ap, 0.0, None, OP.max)
                else:
                    nc.vector.tensor_scalar(m[:], pre_ap, shcol, 0.0, OP.add,
                                            OP.min)
                    nc.vector.tensor_scalar(r[:], pre_ap, shcol, 0.0, OP.add,
                                            OP.max)
                nc.scalar.activation(e[:], m[:], AF.Exp, scale=1.0 / LAM)
                nc.vector.tensor_scalar(e2[:], e[:], LA, -LA, OP.mult, OP.add)
                nc.vector.tensor_add(dst[:], r[:], e2[:])
                return dst

            # ---------------- stage 1+2 layer 1 (bass || alto) ----------
            shb = matvec("b")
            sha = matvec("a")
            h1b = [selu_chain(shb[mt][:], None, 128, 1, f"h1b{mt}")
                   for mt in range(MT)]
            h1a = [selu_chain(aohT[:, mt * P:(mt + 1) * P], sha[mt][:], 128, P,
                              f"h1a{mt}")
                   for mt in range(MT)]

            W1 = P + 1
            hb1 = dp.tile([HS, W1], f32, tag="hb1")
            for mt in range(MT):
                nc.gpsimd.dma_start(hb1[mt * 128:(mt + 1) * 128, 0:P],
                                    h1a[mt][:])
                nc.gpsimd.dma_start(hb1[mt * 128:(mt + 1) * 128, P:W1],
                                    h1b[mt][:])
            ghb1 = dp.tile([H, W1], f32, tag="ghb1")
            nc.gpsimd.collective_compute(
                "AllGather", OP.bypass, replica_groups=RG,
                ins=[hb1[:].opt()], outs=[ghb1[:].opt()])
            H1T = wp.tile([128, KT2 * W1], f32, tag="HT", bufs=2, name="H1T")
            nc.gpsimd.dma_start(
                H1T[:].rearrange("p (kt w) -> p kt w", w=W1),
                ghb1[:].rearrange("(kt p) w -> p kt w", p=128))

            # ------------- stage 3 layer-1 mat-vec (independent) ---------
            sht = matvec("t")

            # ---------------- stage 1+2 layer 2 + logits ----------------
            h2a = []
            h2b = []
            for mt in range(MT):
                pya = pp_l1.tile([128, P], f32, tag="l2", name="pya")
                for kt in range(KT2):
                    nc.tensor.matmul(
                        pya[:],
                        w2s["a"][:, kt * HS + mt * 128:kt * HS + (mt + 1) * 128],
                        H1T[:, kt * W1:kt * W1 + P],
                        start=(kt == 0), stop=False)
                nc.tensor.matmul(pya[:], b2s["a"][:1, mt * 128:(mt + 1) * 128],
                                 onr[:1, :P], start=False, stop=True)
                h2a.append(selu_chain(pya[:], None, 128, P, f"h2a{mt}"))
                pyb = pp_tp.tile([128, 1], f32, tag="tp", name="pyb")
                for kt in range(KT2):
                    nc.tensor.matmul(
                        pyb[:],
                        w2s["b"][:, kt * HS + mt * 128:kt * HS + (mt + 1) * 128],
                        H1T[:, kt * W1 + P:kt * W1 + W1],
                        start=(kt == 0), stop=False)
                nc.tensor.matmul(pyb[:], b2s["b"][:1, mt * 128:(mt + 1) * 128],
                                 onr[:1, :1], start=False, stop=True)
                h2b.append(selu_chain(pyb[:], None, 128, 1, f"h2b{mt}"))

            # AllGather h2 (fused alto+bass), then local logits vs full w3
            hb2 = dp.tile([HS, W1], f32, tag="hb2")
            for mt in range(MT):
                nc.gpsimd.dma_start(hb2[mt * 128:(mt + 1) * 128, 0:P],
                                    h2a[mt][:])
                nc.gpsimd.dma_start(hb2[mt * 128:(mt + 1) * 128, P:W1],
                                    h2b[mt][:])
            ghb2 = dp.tile([H, W1], f32, tag="ghb2")
            nc.gpsimd.collective_compute(
                "AllGather", OP.bypass, replica_groups=RG,
                ins=[hb2[:].opt()], outs=[ghb2[:].opt()])
            H2T = wp.tile([128, KT2 * W1], f32, tag="HT", bufs=2, name="H2T")
            nc.gpsimd.dma_start(
                H2T[:].rearrange("p (kt w) -> p kt w", w=W1),
                ghb2[:].rearrange("(kt p) w -> p kt w", p=128))

            plg_a = pp_sel.tile([P, P], f32, tag="ps", name="plg_a")
            for kt in range(KT2):
                nc.tensor.matmul(plg_a[:], H2T[:, kt * W1:kt * W1 + P],
                                 w3s["a"][:, kt * P:(kt + 1) * P],
                                 start=(kt == 0), stop=False)
            nc.tensor.matmul(plg_a[:], onr[:1, :P], b3s["a"][:1, :],
                             start=False, stop=True)
            plg_b = pp_tp.tile([1, P], f32, tag="tp", name="plg_b")
            for kt in range(KT2):
                nc.tensor.matmul(plg_b[:], H2T[:, kt * W1 + P:kt * W1 + W1],
                                 w3s["b"][:, kt * P:(kt + 1) * P],
                                 start=(kt == 0), stop=False)
            nc.tensor.matmul(plg_b[:], onr[:1, :1], b3s["b"][:1, :],
                             start=False, stop=True)

            # fused softmax: alto rows 0..61 at base 0, bass row copied to 64
            NR = 65
            lgcat = wp.tile([NR, P], f32, tag="lgcat")
            nc.vector.memset(lgcat[:], 0.0)
            nc.vector.tensor_copy(lgcat[:P, :], plg_a[:])
            nc.vector.tensor_copy(lgcat[64:NR, :], plg_b[:])
            nm = wp.tile([NR, 1], f32, tag="nm")
            nc.vector.tensor_reduce(nm[:], lgcat[:], axis=AX.X, op=OP.max,
                                    negate=True)
            E = wp.tile([NR, P], f32, tag="E")
            ssum = wp.tile([NR, 1], f32, tag="ssum")
            nc.scalar.activation(E[:], lgcat[:], AF.Exp, bias=nm[:],
                                 accum_out=ssum[:])
            rec = wp.tile([NR, 1], f32, tag="rec")
            nc.vector.reciprocal(rec[:], ssum[:])
            erow = wp.tile([1, P], f32, tag="erow")
            nc.vector.tensor_copy(erow[:], E[64:NR, :])
            rc62 = wp.tile([1, 1], f32, tag="rc62")
            nc.vector.tensor_copy(rc62[:], rec[64:NR, 0:1])
            ptp2 = pp_tp.tile([P, 1], f32, tag="tp", name="ptp2")
            nc.tensor.transpose(ptp2[:], erow[:1, :], idn[:1, :1])
            pbc = pp_tp.tile([P, 1], f32, tag="tp", name="pbc")
            nc.tensor.matmul(pbc[:], onr[:1, :P], rc62[:1, :1],
                             start=True, stop=True)
            v1 = wp.tile([P, 1], f32, tag="v1")
            nc.vector.tensor_mul(v1[:], ptp2[:], rec[:P, :])
            v = wp.tile([P, 1], f32, tag="v")
            nc.vector.tensor_mul(v[:], v1[:], pbc[:])
            # anchor the warmup collective so it isn't dead code
            nc.vector.scalar_tensor_tensor(v[:], wg[:P, 0:1], 0.0, v[:],
                                           OP.mult, OP.add)
            PA = wp.tile([P, P], f32, tag="PA")
            nc.vector.tensor_scalar(PA[:], E[:P, :], v[:], None, OP.mult)
            nc.scalar.dma_start(pa_out[:], PA[:])

            # ---------------- on-device top-62 selection ----------------
            # (1) broadcast flat scores to all partitions: R[i, e] = PA_flat[e]
            paf = dp.tile([P, P], f32, tag="paf")
            nc.gpsimd.dma_start(paf[:], PA[:])
            R = wp.tile([P, FLAT], f32, tag="R")
            nc.gpsimd.dma_start(
                R[:],
                paf[:].rearrange("a b -> (a b)")[None, :].broadcast_to(
                    [P, FLAT]))
            # (2) initial bracket: lo = 0, hi = max * 1.00001
            rmx = wp.tile([P, 1], f32, tag="rmx")
            nc.vector.tensor_reduce(rmx[:], PA[:], axis=AX.X, op=OP.max)
            prx = pp_tp.tile([1, P], f32, tag="tp", name="prx")
            nc.tensor.transpose(prx[:], rmx[:], idn[:P, :P])
            rxr = wp.tile([1, P], f32, tag="rxr")
            nc.vector.tensor_copy(rxr[:], prx[:])
            vmx = wp.tile([1, 1], f32, tag="vmx")
            nc.vector.tensor_reduce(vmx[:], rxr[:], axis=AX.X, op=OP.max)
            nc.vector.tensor_scalar(vmx[:], vmx[:], 1.00001, None, OP.mult)
            phi = pp_tp.tile([P, 1], f32, tag="tp", name="phi")
            nc.tensor.matmul(phi[:], onr[:1, :P], vmx[:1, :1], start=True,
                             stop=True)
            hi = wp.tile([P, 1], f32, tag="hi")
            nc.vector.tensor_copy(hi[:], phi[:])
            lo = wp.tile([P, 1], f32, tag="lo")
            nc.vector.memset(lo[:], 0.0)
            tstar = wp.tile([P, 1], f32, tag="tstar")
            nc.vector.memset(tstar[:], 0.0)
            sgn = wp.tile([P, FLAT], f32, tag="sgn")
            BIG = 1.0e30

            def preduce(vec_ap, op, name):
                # [P,1] -> scalar [1,1] via transpose + free reduce
                pt_ = pp_tp.tile([1, P], f32, tag="tp", name=f"pt_{name}")
                nc.tensor.transpose(pt_[:], vec_ap, idn[:P, :P])
                row = tp.tile([1, P], f32, tag="prow", name="prow")
                nc.vector.tensor_copy(row[:], pt_[:])
                sc_ = tp.tile([1, 1], f32, tag="pscl", name="pscl")
                nc.vector.tensor_reduce(sc_[:], row[:], axis=AX.X, op=op)
                return sc_

            def bcast_col(scalar_ap, name):
                pb_ = pp_tp.tile([P, 1], f32, tag="tp", name=f"pb_{name}")
                nc.tensor.matmul(pb_[:], onr[:1, :P], scalar_ap, start=True,
                                 stop=True)
                return pb_

            for rnd in range(3):
                # probes t_i = lo + (i+1)*(hi-lo)/63
                stp = tp.tile([P, 1], f32, tag="stp", name="stp")
                nc.vector.tensor_sub(stp[:], hi[:], lo[:])
                nc.vector.tensor_scalar(stp[:], stp[:], 1.0 / 63.0, None,
                                        OP.mult)
                tcol = tp.tile([P, 1], f32, tag="tcol", name="tcol")
                nc.vector.scalar_tensor_tensor(tcol[:], ioc1[:], stp[:],
                                               lo[:], OP.mult, OP.add)
                nbt = tp.tile([P, 1], f32, tag="nbt", name="nbt")
                nc.vector.tensor_scalar(nbt[:], tcol[:], -1.0, None, OP.mult)
                ssg = tp.tile([P, 1], f32, tag="ssg", name="ssg")
                nc.scalar.activation(sgn[:], R[:], AF.Sign, bias=nbt[:],
                                     accum_out=ssg[:])
                cnt = tp.tile([P, 1], f32, tag="cnt", name="cnt")
                nc.vector.tensor_scalar(cnt[:], ssg[:], 0.5, FLAT / 2.0,
                                        OP.mult, OP.add)
                # lo update: largest probe with cnt >= 62.75
                mlo = tp.tile([P, 1], f32, tag="mlo", name="mlo")
                nc.vector.tensor_scalar(mlo[:], cnt[:], 62.75, None, OP.is_ge)
                lc = tp.tile([P, 1], f32, tag="lc", name="lc")
                nc.vector.tensor_mul(lc[:], tcol[:], mlo[:])
                lmax = preduce(lc[:], OP.max, f"lm{rnd}")
                plo = bcast_col(lmax[:1, :1], f"lo{rnd}")
                nc.vector.tensor_max(lo[:], lo[:], plo[:])
                # hi update: smallest probe with cnt <= 62.25
                mhi = tp.tile([P, 1], f32, tag="mhi", name="mhi")
                nc.vector.tensor_scalar(mhi[:], cnt[:], 62.25, None, OP.is_le)
                hc = tp.tile([P, 1], f32, tag="hc", name="hc")
                nc.vector.tensor_mul(hc[:], tcol[:], mhi[:])
                hc2 = tp.tile([P, 1], f32, tag="hc2", name="hc2")
                nc.vector.tensor_scalar(hc2[:], mhi[:], -BIG, BIG, OP.mult,
                                        OP.add)
                nc.vector.tensor_add(hc[:], hc[:], hc2[:])
                hmin = preduce(hc[:], OP.min, f"hm{rnd}")
                phi2 = bcast_col(hmin[:1, :1], f"hi{rnd}")
                nc.vector.tensor_tensor(hi[:], hi[:], phi2[:], OP.min)
                # t* candidates: probes with cnt == 62 (within 0.25)
                c62 = tp.tile([P, 1], f32, tag="c62", name="c62")
                nc.vector.tensor_scalar(c62[:], cnt[:], -62.0, None, OP.add)
                sq = tp.tile([P, 1], f32, tag="sq", name="sq")
                nc.vector.tensor_mul(sq[:], c62[:], c62[:])
                meq = tp.tile([P, 1], f32, tag="meq", name="meq")
                nc.vector.tensor_scalar(meq[:], sq[:], 0.07, None, OP.is_le)
                tc2 = tp.tile([P, 1], f32, tag="tc2", name="tc2")
                nc.vector.tensor_mul(tc2[:], tcol[:], meq[:])
                tmax = preduce(tc2[:], OP.max, f"ts{rnd}")
                pts = bcast_col(tmax[:1, :1], f"tx{rnd}")
                nc.vector.tensor_max(tstar[:], tstar[:], pts[:])

            # (3) mask / compaction ranks / one-hots (verified scheme)
            mask = wp.tile([P, P], f32, tag="mask")
            nc.vector.tensor_scalar(mask[:], PA[:], tstar[:], None, OP.is_gt)
            pmT = pp_sel.tile([P, P], f32, tag="ps", name="pmT")
            nc.tensor.transpose(pmT[:], mask[:], idn[:P, :P])
            mT = wp.tile([P, P], f32, tag="mT")
            nc.vector.tensor_copy(mT[:], pmT[:])
            prc = pp_sel.tile([P, P], f32, tag="ps", name="prc")
            nc.tensor.matmul(prc[:], mT[:], lt[:], start=True, stop=True)
            rcm = wp.tile([P, P], f32, tag="rcm")
            nc.vector.tensor_copy(rcm[:], prc[:])
            pro = pp_tp.tile([1, P], f32, tag="tp", name="pro")
            nc.tensor.matmul(pro[:], rcm[:, P - 1:P], slt[:], start=True,
                             stop=True)
            ror = wp.tile([1, P], f32, tag="ror")
            nc.vector.tensor_copy(ror[:], pro[:])
            proc = pp_tp.tile([P, 1], f32, tag="tp", name="proc")
            nc.tensor.transpose(proc[:], ror[:1, :], idn[:1, :1])
            roc = wp.tile([P, 1], f32, tag="roc")
            nc.vector.tensor_copy(roc[:], proc[:])
            re_ = wp.tile([P, 1], f32, tag="re")
            nc.vector.tensor_add(re_[:], roc[:], rcm[:, P - 1:P])
            g1 = tp.tile([P, P], f32, tag="selu_m", name="g1")
            nc.vector.tensor_scalar(g1[:], iof[:], roc[:], None, OP.is_ge)
            g2 = tp.tile([P, P], f32, tag="selu_r", name="g2")
            nc.vector.tensor_scalar(g2[:], iof[:], re_[:], None, OP.is_lt)
            bb = wp.tile([P, P], f32, tag="bb")
            nc.vector.tensor_mul(bb[:], g1[:], g2[:])
            t1 = tp.tile([P, P], f32, tag="selu_e", name="t1")
            nc.vector.tensor_scalar(t1[:], rcm[:], roc[:], None, OP.add)
            t2 = tp.tile([P, P], f32, tag="selu_e2", name="t2")
            nc.vector.tensor_mul(t2[:], t1[:], mask[:])
            t3 = tp.tile([P, P], f32, tag="selu_m", name="t3")
            nc.vector.tensor_scalar(t3[:], t2[:], -1.0, None, OP.add)
            idx = wp.tile([64, P], i16, tag="idx")
            nc.vector.memset(idx[:], -1)
            nc.vector.tensor_copy(idx[:P, :], t3[:])
            scx = wp.tile([64, 64], bf16, tag="scx")
            nc.gpsimd.local_scatter(scx[:], iofb[:], idx[:], channels=64,
                                    num_elems=64, num_idxs=P)
            pas = pp_tp.tile([1, P], f32, tag="tp", name="pas")
            nc.tensor.matmul(pas[:], ocb[:], scx[:P, :P], start=True,
                             stop=True)
            asr = wp.tile([1, P], f32, tag="asr")
            nc.vector.tensor_copy(asr[:], pas[:])
            pab = pp_sel.tile([P, P], f32, tag="ps", name="pab")
            nc.tensor.matmul(pab[:], onr[:1, :P], asr[:1, :], start=True,
                             stop=True)
            ba = wp.tile([P, P], f32, tag="ba")
            nc.vector.tensor_scalar(ba[:], pab[:], ioc[:], None, OP.is_equal)
            pz = pp_sel.tile([P, P], f32, tag="ps", name="pz")
            nc.tensor.matmul(pz[:], bb[:], PA[:], start=True, stop=True)
            pbat = pp_sel.tile([P, P], f32, tag="ps", name="pbat")
            nc.tensor.transpose(pbat[:], ba[:], idn[:P, :P])
            bat = wp.tile([P, P], f32, tag="bat")
            nc.vector.tensor_copy(bat[:], pbat[:])
            pmm = tp.tile([P, P], f32, tag="selu_r", name="pmm")
            nc.vector.tensor_mul(pmm[:], pz[:], bat[:])
            pcol = wp.tile([P, 1], f32, tag="pcol")
            nc.vector.tensor_reduce(pcol[:], pmm[:], axis=AX.X, op=OP.add)

            # ---------------- stage 3 (tenor) ----------------
            h1t = []
            for mt in range(MT):
                pg = pp_l1.tile([128, P], f32, tag="l2", name="pg")
                nc.tensor.matmul(pg[:], tohb[:, mt * 128:(mt + 1) * 128],
                                 bb[:], start=True, stop=False)
                nc.tensor.matmul(pg[:], toha[:, mt * 128:(mt + 1) * 128],
                                 ba[:], start=False, stop=True)
                h1t.append(selu_chain(pg[:], sht[mt][:], 128, P, f"h1t{mt}"))

            ht1 = dp.tile([HS, P], f32, tag="ht1")
            for mt in range(MT):
                nc.gpsimd.dma_start(ht1[mt * 128:(mt + 1) * 128, :],
                                    h1t[mt][:])
            ght = dp.tile([H, P], f32, tag="ght")
            nc.gpsimd.collective_compute(
                "AllGather", OP.bypass, replica_groups=RG,
                ins=[ht1[:].opt()], outs=[ght[:].opt()])
            H1tT = wp.tile([128, KT2 * P], f32, tag="HT", bufs=2, name="H1tT")
            nc.gpsimd.dma_start(
                H1tT[:].rearrange("p (kt w) -> p kt w", w=P),
                ght[:].rearrange("(kt p) w -> p kt w", p=128))

            h2t = []
            for mt in range(MT):
                pyt = pp_l1.tile([128, P], f32, tag="l2", name="pyt")
                for kt in range(KT2):
                    nc.tensor.matmul(
                        pyt[:],
                        w2s["t"][:, kt * HS + mt * 128:kt * HS + (mt + 1) * 128],
                        H1tT[:, kt * P:(kt + 1) * P],
                        start=(kt == 0), stop=False)
                nc.tensor.matmul(pyt[:], b2s["t"][:1, mt * 128:(mt + 1) * 128],
                                 onr[:1, :P], start=False, stop=True)
                h2t.append(selu_chain(pyt[:], None, 128, P, f"h2t{mt}"))

            ht2 = dp.tile([HS, P], f32, tag="ht2")
            for mt in range(MT):
                nc.gpsimd.dma_start(ht2[mt * 128:(mt + 1) * 128, :],
                                    h2t[mt][:])
            ght2 = dp.tile([H, P], f32, tag="ght2")
            nc.gpsimd.collective_compute(
                "AllGather", OP.bypass, replica_groups=RG,
                ins=[ht2[:].opt()], outs=[ght2[:].opt()])
            H2tT = wp.tile([128, KT2 * P], f32, tag="HT", bufs=2, name="H2tT")
            nc.gpsimd.dma_start(
                H2tT[:].rearrange("p (kt w) -> p kt w", w=P),
                ght2[:].rearrange("(kt p) w -> p kt w", p=128))

            plg_t = pp_sel.tile([P, P], f32, tag="ps", name="plg_t")
            for kt in range(KT2):
                nc.tensor.matmul(plg_t[:], H2tT[:, kt * P:(kt + 1) * P],
                                 w3s["t"][:, kt * P:(kt + 1) * P],
                                 start=(kt == 0), stop=False)
            nc.tensor.matmul(plg_t[:], onr[:1, :P], b3s["t"][:1, :],
                             start=False, stop=True)
            S3 = wp.tile([P, P], f32, tag="S3")
            nc.vector.tensor_copy(S3[:], plg_t[:])
            nm3 = wp.tile([P, 1], f32, tag="nm3")
            nc.vector.tensor_reduce(nm3[:], S3[:], axis=AX.X, op=OP.max,
                                    negate=True)
            E3 = wp.tile([P, P], f32, tag="E3")
            ssum3 = wp.tile([P, 1], f32, tag="ssum3")
            nc.scalar.activation(E3[:], S3[:], AF.Exp, bias=nm3[:],
                                 accum_out=ssum3[:])
            rec3 = wp.tile([P, 1], f32, tag="rec3")
            nc.vector.reciprocal(rec3[:], ssum3[:])
            vv = wp.tile([P, 1], f32, tag="vv")
            nc.vector.tensor_mul(vv[:], rec3[:], pcol[:])
            PT = wp.tile([P, P], f32, tag="PT")
            nc.vector.tensor_scalar(PT[:], E3[:], vv[:], None, OP.mult)
            nc.scalar.dma_start(pt_out[:], PT[:])

    nc.compile()
    return nc


_NC_CACHE = None


def _get_nc():
    global _NC_CACHE
    if _NC_CACHE is None:
        _NC_CACHE = _build()
    return _NC_CACHE


def _prep_inputs(inputs):
    lam = np.float32(LAM)
    x = np.asarray(inputs["inputs_bass"], np.float32)

    def w1timg(w):
        # [D, 256] -> transposed image [128, MT*D]: img[p, mt*D+k] = w[k, mt*128+p]
        wt = np.ascontiguousarray(w.T)              # [256, D]
        return np.ascontiguousarray(
            wt.reshape(MT, 128, D).transpose(1, 0, 2).reshape(128, MT * D))

    def w2img(w):
        return np.ascontiguousarray(
            w.reshape(KT2, 128, HS).transpose(1, 0, 2).reshape(128, KT2 * HS))

    def w3img(w):
        # [2048, P] -> [128, KT2*P]
        return np.ascontiguousarray(
            w.reshape(KT2, 128, P).transpose(1, 0, 2).reshape(128, KT2 * P))

    def mtimg(w):
        # [256, P] -> [128, MT*P]
        return np.ascontiguousarray(
            w.reshape(MT, 128, P).transpose(1, 0, 2).reshape(128, MT * P))

    base = {
        "ident": np.eye(128, dtype=np.float32),
        "LTc": (np.arange(P)[:, None] <= np.arange(P)[None, :]).astype(np.float32),
        "SLTc": (np.arange(P)[:, None] < np.arange(P)[None, :]).astype(np.float32),
        "iotaF": np.broadcast_to(np.arange(P, dtype=np.float32), (P, P)).copy(),
        "iotaC": np.arange(P, dtype=np.float32)[:, None].copy(),
        "iotaC1": (np.arange(P, dtype=np.float32)[:, None] + 1.0).copy(),
        "onesR": np.ones((1, HS), np.float32),
        "onesCbf": np.ones((P, 1), ml_dtypes.bfloat16),
        "iotaFbf": np.broadcast_to(
            np.arange(P, dtype=ml_dtypes.bfloat16), (64, P)).copy(),
        "warm": np.zeros((16, 32), np.float32),
        "xr": x[None, :].copy(),
    }
    W = {k: np.asarray(v, np.float32) for k, v in inputs.items()}
    in_maps = []
    for c in range(NCORES):
        cols = slice(HS * c, HS * (c + 1))
        m = dict(base)
        for s in "bat":
            m[f"{s}w1t"] = w1timg(lam * W[f"{s}w1"][:D, cols])
            m[f"{s}w2i"] = w2img(lam * W[f"{s}w2"][:, cols])
            m[f"{s}w3i"] = w3img(W[f"{s}w3"])
            m[f"{s}b1c"] = np.ascontiguousarray(
                (lam * W[f"{s}b1"][cols]).reshape(MT, 128).T)
            m[f"{s}b2r"] = (lam * W[f"{s}b2"][cols])[None, :].copy()
            m[f"{s}b3r"] = W[f"{s}b3"][None, :].copy()
        m["aohT"] = mtimg(np.ascontiguousarray(
            (lam * W["aw1"][D:D + P, cols]).T))
        m["tohb"] = np.ascontiguousarray(lam * W["tw1"][D:D + P, cols])
        m["toha"] = np.ascontiguousarray(lam * W["tw1"][D + P:D + 2 * P, cols])
        in_maps.append(m)
    return in_maps


def _postprocess(pa, pt):
    flat = pa.reshape(-1)
    order = np.argsort(-flat, kind="stable")[:P]
    sel = np.sort(order)                  # device rank order = flat position
    j_sel = sel // P
    a_sel = sel % P
    flat3 = pt.reshape(-1)
    idx3 = np.argsort(-flat3, kind="stable")[:P]
    row = idx3 // P
    out = np.stack([
        flat3[idx3],
        j_sel[row].astype(np.float32),
        a_sel[row].astype(np.float32),
        (idx3 % P).astype(np.float32),
    ], axis=1)
    return out


def run(inputs, trace=False):
    nc = _get_nc()
    in_maps = _prep_inputs(inputs)
    res = bass_utils.run_bass_kernel_spmd(
        nc, in_maps, core_ids=list(range(NCORES)), trace=trace)
    r0 = res.results[0]
    out = _postprocess(r0["pa_out"], r0["pt_out"])
    return out, res.exec_time_ns


def kernel(**inputs) -> np.ndarray:
    out, _ = run(inputs, trace=False)
    return out
